# revision 52
# baseline (speedup 1.0000x reference)
"""GraphTransformerEncoder (8-layer TransformerConv + BN + ReLU + mean-pool)
on 8 Trainium2 NeuronCores via Bass/Tile.

Sharding: graph-parallel. Core c owns graphs [8c, 8c+8) -> a contiguous node
range (batch is sorted). Edges are owned by the core of their dst node, sorted
by dst, and packed into per-128-node-block chunk lists with per-block counts
fitted to the data (max over cores, so the single SPMD program works for all).
Per layer each core computes K/V projections, AllGathers the K|V table (bf16),
computes Q/root projections while the collective runs, DMA-gathers K|V rows
for its edges' src nodes, computes the edge softmax via segment-indicator
matmuls (indicator transposed table host-precomputed and SBUF-resident), and
applies BN (global stats via a tiny AllReduce) + ReLU, then mean-pools.
"""

import numpy as np
import ml_dtypes

import concourse.bass as bass
import concourse.bacc as bacc
import concourse.mybir as mybir
import concourse.tile as tile
from concourse import library_config
from contextlib import ExitStack

BF = mybir.dt.bfloat16
F8 = mybir.dt.float8e4
F32 = mybir.dt.float32
I16 = mybir.dt.int16
AF = mybir.ActivationFunctionType

# problem constants
N, E, F, H, C, L, B = 10000, 160000, 128, 8, 64, 8, 64
D = H * C  # 512
BN_EPS = 1e-5

NCORE = 8
GPC = B // NCORE        # graphs per core = 8
GI = 512                # indices per dma_gather (4 chunks)
GBUFS = 5               # gather tiles in flight
RW = 1536               # K|V row bytes: 1KB bf16 K + 512B fp8 V


def _to_bf(a):
    return np.asarray(a, dtype=np.float32).astype(ml_dtypes.bfloat16)


def _build_nc(NT, CH, GB):
    """Build the SPMD program. NT: node blocks per core; CH[m]: chunks per
    block (128 edge slots each); GB[m]: gathers per block (512 slots each)."""
    NLOC = NT * 128
    KVROWS = NCORE * NLOC
    CHT = sum(CH)
    NGA = sum(GB)

    nc = bacc.Bacc("TRN2", num_devices=NCORE,
                  target_bir_lowering=False, debug=False)
    rg = [list(range(NCORE))]

    # ---- I/O -----------------------------------------------------------
    XT = nc.dram_tensor("XT", [128, NLOC], BF, kind="ExternalInput")
    W0 = nc.dram_tensor("W0", [128, 4 * 512], BF, kind="ExternalInput")
    WR = nc.dram_tensor("WR", [7 * 2048, 512], BF, kind="ExternalInput")
    BIAS = nc.dram_tensor("BIAS", [1, 8 * 2048], BF, kind="ExternalInput")
    GAM = nc.dram_tensor("GAM", [1, 8 * 512], F32, kind="ExternalInput")
    BET = nc.dram_tensor("BET", [1, 8 * 512], F32, kind="ExternalInput")
    IDX = nc.dram_tensor("IDX", [128, NGA * (GI // 16)], I16, kind="ExternalInput")
    STC = nc.dram_tensor("STC", [128, CHT * 128], BF, kind="ExternalInput")
    STT = nc.dram_tensor("STT", [128, CHT * 128], BF, kind="ExternalInput")
    ONES1 = nc.dram_tensor("ONES1", [1, 128], BF, kind="ExternalInput")
    IDENTF = nc.dram_tensor("IDENTF", [128, 128], F32, kind="ExternalInput")
    IDENTB = nc.dram_tensor("IDENTB", [128, 128], BF, kind="ExternalInput")
    MASK = nc.dram_tensor("MASK", [128, NT], F32, kind="ExternalInput")
    SPOOL = nc.dram_tensor("SPOOL", [128, NT * GPC], BF, kind="ExternalInput")
    CNTR = nc.dram_tensor("CNTR", [GPC, 1], F32, kind="ExternalInput")
    OUT = nc.dram_tensor("POOLED", [GPC, L * 512], F32, kind="ExternalOutput")

    with tile.TileContext(nc) as tc, ExitStack() as ctx:
        sb1 = ctx.enter_context(tc.tile_pool(name="sb1", bufs=1))
        sbh = ctx.enter_context(tc.tile_pool(name="sbh", bufs=2))
        sbw = ctx.enter_context(tc.tile_pool(name="sbw", bufs=2))
        sbs = ctx.enter_context(tc.tile_pool(name="sbs", bufs=3))
        sbg = ctx.enter_context(tc.tile_pool(name="sbg", bufs=GBUFS))
        sbm = ctx.enter_context(tc.tile_pool(name="sbm", bufs=2))
        ps = ctx.enter_context(tc.tile_pool(name="ps", bufs=1, space="PSUM"))
        dram = ctx.enter_context(tc.tile_pool(name="dram", bufs=2, space="DRAM"))

        def load1(src, shape, dtype, name):
            t = sb1.tile(shape, dtype, name=name)
            nc.sync.dma_start(out=t[:], in_=src[:])
            return t

        ones1 = load1(ONES1, [1, 128], BF, "ones1")
        identf = load1(IDENTF, [128, 128], F32, "identf")
        identb = load1(IDENTB, [128, 128], BF, "identb")
        idx_sb = load1(IDX, [128, NGA * (GI // 16)], I16, "idx_sb")
        mask_sb = load1(MASK, [128, NT], F32, "mask_sb")
        spool_sb = load1(SPOOL, [128, NT * GPC], BF, "spool_sb")
        cntr_sb = load1(CNTR, [GPC, 1], F32, "cntr_sb")

        czero = sb1.tile([128, 1], F32, name="czero")
        nc.vector.memset(czero[:], 0.0)
        ceps = sb1.tile([128, 1], F32, name="ceps")
        nc.vector.memset(ceps[:], BN_EPS)
        nc.const_aps.aps[(F32, 0.0)] = czero[:]
        nc.const_aps.aps[(F32, BN_EPS)] = ceps[:]

        nc.gpsimd.load_library(library_config.mlp)

        h_cur = sbh.tile([128, 4, NLOC], BF, tag="h", name="h0")
        nc.sync.dma_start(out=h_cur[:, 0, :], in_=XT[:, :])

        # NaN guard: first-layer gathers skip -1 slots, leaving stale SBUF.
        for i in range(GBUFS):
            gz = sbg.tile([128, 4, RW], F8, tag="g", name=f"gz{i}")
            nc.vector.memset(gz[:], 0.0)

        def emit_pool(lp, h):
            """Mean-pool layer lp's output h (feature-major) into OUT."""
            poolp = ps.tile([8, 512], F32, tag="stat", bufs=1,
                            name=f"poolp{lp}")
            for m in range(NT):
                hnm = sbm.tile([128, 512], BF, tag="hnm", bufs=2,
                               name=f"hnm{lp}_{m}")
                for kc in range(4):
                    tp2 = ps.tile([128, 128], BF, tag="tp2", bufs=1,
                                  name=f"tp2{lp}_{m}_{kc}")
                    nc.tensor.transpose(
                        tp2[:], h[:, kc, m * 128:(m + 1) * 128], identb[:])
                    nc.scalar.activation(hnm[:, kc * 128:(kc + 1) * 128],
                                         tp2[:], AF.Copy)
                nc.tensor.matmul(poolp[:],
                                 lhsT=spool_sb[:, m * GPC:(m + 1) * GPC],
                                 rhs=hnm[:], start=(m == 0),
                                 stop=(m == NT - 1))
            pool_sb = sbs.tile([GPC, 512], F32, tag="poolsb", bufs=2,
                               name=f"pool{lp}")
            nc.scalar.activation(pool_sb[:], poolp[:], AF.Identity,
                                 scale=cntr_sb[:, 0:1])
            nc.sync.dma_start(out=OUT[:, lp * 512:(lp + 1) * 512],
                              in_=pool_sb[:])

        for l in range(L):
            KIN = 1 if l == 0 else 4

            w_sb = sbw.tile([128, 4 * KIN, 512], BF, tag="w", name=f"w{l}")
            if l == 0:
                nc.sync.dma_start(
                    out=w_sb[:], in_=W0[:, :].rearrange("p (c n) -> p c n", c=4))
            else:
                nc.sync.dma_start(
                    out=w_sb[:],
                    in_=WR[(l - 1) * 2048: l * 2048, :].rearrange(
                        "(c p) n -> p c n", p=128))

            bias_sb = sbs.tile([1, 2048], BF, tag="bias", bufs=2,
                               name=f"bias{l}")
            nc.sync.dma_start(out=bias_sb[:],
                              in_=BIAS[0:1, l * 2048:(l + 1) * 2048])
            gam_sb = sbs.tile([1, 512], F32, tag="gam", bufs=2, name=f"gam{l}")
            nc.sync.dma_start(out=gam_sb[:], in_=GAM[0:1, l * 512:(l + 1) * 512])
            bet_sb = sbs.tile([1, 512], F32, tag="bet", bufs=2, name=f"bet{l}")
            nc.sync.dma_start(out=bet_sb[:], in_=BET[0:1, l * 512:(l + 1) * 512])

            kv_loc = dram.tile([NLOC, RW], F8, tag="kvloc", name=f"kvloc{l}")
            kv_full = dram.tile([KVROWS, RW], F8, tag="kvfull",
                                addr_space="Shared", name=f"kvfull{l}")

            # -- phase A1: K,V projections -> kv_loc; AllGather in two halves
            # so the first collective overlaps the second half's projections
            for m in range(NT):
                kv_sb = sbm.tile([128, RW], F8, tag="kvp", bufs=2,
                                 name=f"kv{l}_{m}")
                for pr in (1, 2):  # 1=k 2=v
                    pp = ps.tile([128, 2, 512], F32, tag="qd", bufs=2,
                                 name=f"pp{l}_{m}_{pr}")
                    for kc in range(KIN):
                        nc.tensor.matmul(
                            pp[:, 0, :], lhsT=h_cur[:, kc, m * 128:(m + 1) * 128],
                            rhs=w_sb[:, pr * KIN + kc, :],
                            start=(kc == 0), stop=False)
                    nc.tensor.matmul(
                        pp[:, 0, :], lhsT=ones1[:],
                        rhs=bias_sb[0:1, pr * 512:(pr + 1) * 512],
                        start=False, stop=True)
                    if pr == 1:   # K half, bf16
                        nc.scalar.activation(
                            kv_sb[:, 0:1024].bitcast(BF), pp[:, 0, :], AF.Copy)
                    else:         # V half, fp8e4
                        nc.scalar.activation(
                            kv_sb[:, 1024:RW], pp[:, 0, :], AF.Copy)
                nc.sync.dma_start(out=kv_loc[m * 128:(m + 1) * 128, :],
                                  in_=kv_sb[:])

            nc.gpsimd.collective_compute(
                "AllGather", mybir.AluOpType.bypass, replica_groups=rg,
                ins=[kv_loc[:].opt()], outs=[kv_full[:].opt()])

            # previous layer's pooling, deferred into the AllGather window
            if l > 0:
                emit_pool(l - 1, h_cur)

            # -- phase A2 (overlaps AllGather): Q + root projections
            Q_sb = sbm.tile([128, NT, 512], BF, tag="q", bufs=1, name=f"q{l}")
            pre_sb = sbm.tile([128, NT, 512], F32, tag="pre", bufs=1,
                              name=f"pre{l}")
            for m in range(NT):
                for pr in (0, 3):  # 0=q 3=root
                    pp = ps.tile([128, 2, 512], F32, tag="qd", bufs=2,
                                 name=f"qr{l}_{m}_{pr}")
                    for kc in range(KIN):
                        nc.tensor.matmul(
                            pp[:, 0, :], lhsT=h_cur[:, kc, m * 128:(m + 1) * 128],
                            rhs=w_sb[:, pr * KIN + kc, :],
                            start=(kc == 0), stop=False)
                    nc.tensor.matmul(
                        pp[:, 0, :], lhsT=ones1[:],
                        rhs=bias_sb[0:1, pr * 512:(pr + 1) * 512],
                        start=False, stop=True)
                    if pr == 0:
                        nc.scalar.activation(Q_sb[:, m, :], pp[:, 0, :], AF.Copy)
                    else:
                        nc.scalar.activation(pre_sb[:, m, :], pp[:, 0, :],
                                             AF.Copy)

            # -- phase B: edge stage
            # rows 0 / 32: sum / sum-of-squares (matmul out base partition
            # must be 0, 32, or 64)
            stat_ps = ps.tile([33, 512], F32, tag="stat", bufs=1,
                              name=f"stat{l}")
            ch0 = 0
            ga0 = 0
            for m in range(NT):
                # stream both indicator orientations for this block (tiny,
                # on the otherwise-idle regular DMA queues)
                stb = sbs.tile([128, CH[m] * 128], BF, tag="stb", bufs=2,
                               name=f"stb{l}_{m}")
                nc.sync.dma_start(
                    out=stb[:], in_=STC[:, ch0 * 128:(ch0 + CH[m]) * 128])
                sttb = sbs.tile([128, CH[m] * 128], BF, tag="sttb", bufs=2,
                                name=f"sttb{l}_{m}")
                nc.sync.dma_start(
                    out=sttb[:], in_=STT[:, ch0 * 128:(ch0 + CH[m]) * 128])
                acc = ps.tile([128, 512], F32, tag="acc", bufs=1,
                              name=f"acc{l}_{m}")
                den = ps.tile([128, 8], F32, tag="den", bufs=1,
                              name=f"den{l}_{m}")
                for g in range(GB[m]):
                    nch = min(4, CH[m] - 4 * g)
                    gt = sbg.tile([128, 4, RW], F8, tag="g",
                                  name=f"gt{l}_{m}_{g}")
                    ga = ga0 + g
                    nidx = nch * 128  # partial tail gathers move fewer rows
                    nc.gpsimd.dma_gather(
                        gt[:, 0:nch, :], kv_full[:, :],
                        idx_sb[:, ga * (GI // 16):
                               ga * (GI // 16) + nidx // 16],
                        nidx, nidx, RW)
                    lg = sbs.tile([128, 4, 8], F32, tag="lg", bufs=4,
                                  name=f"lg{l}_{ga}")
                    pbf = sbs.tile([128, 4, 8], BF, tag="p", bufs=4,
                                   name=f"p{l}_{ga}")
                    pv = sbm.tile([128, 4, 512], BF, tag="pv", bufs=2,
                                  name=f"pv{l}_{ga}")
                    for g2 in range((nch + 1) // 2):
                        n2 = min(2, nch - 2 * g2)
                        qd = ps.tile([128, 2, 512], F32, tag="qd", bufs=2,
                                     name=f"qd{l}_{ga}_{g2}")
                        for i in range(n2):
                            ci = 4 * g + 2 * g2 + i
                            nc.tensor.matmul(
                                qd[:, i, :],
                                lhsT=sttb[:, ci * 128:(ci + 1) * 128],
                                rhs=Q_sb[:, m, :], start=True, stop=True)
                        nc.scalar.activation(pv[:, 2 * g2:2 * g2 + n2, :],
                                             qd[:, 0:n2, :], AF.Copy)
                    pvv = pv[:, 0:nch, :]
                    # alternate the K-multiply between vector and gpsimd to
                    # split the edge-phase element-wise load
                    mul_eng = nc.vector if (ga % 2 == 0) else nc.gpsimd
                    mul_eng.tensor_mul(pvv, pvv,
                                       gt[:, 0:nch, 0:1024].bitcast(BF))
                    nc.vector.tensor_reduce(
                        lg[:, 0:nch, :],
                        pvv.rearrange("p n (h c) -> p n h c", h=8),
                        mybir.AxisListType.X, mybir.AluOpType.add)
                    nc.scalar.activation(pbf[:, 0:nch, :], lg[:, 0:nch, :],
                                         AF.Exp, scale=0.125)
                    nc.vector.tensor_mul(
                        pv[:, 0:nch, :].rearrange("p n (h c) -> p n h c", h=8),
                        gt[:, 0:nch, 1024:RW].rearrange(
                            "p n (h c) -> p n h c", h=8),
                        pbf[:, 0:nch, :, None].broadcast_to([128, nch, 8, 64]))
                    for cc in range(nch):
                        ci = 4 * g + cc
                        first = (ci == 0)
                        last = (ci == CH[m] - 1)
                        nc.tensor.matmul(acc[:],
                                         lhsT=stb[:, ci * 128:(ci + 1) * 128],
                                         rhs=pv[:, cc, :],
                                         start=first, stop=last)
                        nc.tensor.matmul(den[:],
                                         lhsT=stb[:, ci * 128:(ci + 1) * 128],
                                         rhs=pbf[:, cc, :],
                                         start=first, stop=last)

                # block finalize: normalize, add root (staged in pre_sb), stats
                dsb = sbs.tile([128, 8], F32, tag="dsb", name=f"dsb{l}_{m}")
                nc.scalar.activation(dsb[:], den[:], AF.Copy, bias=1e-16)
                rec = sbs.tile([128, 8], F32, tag="rec", name=f"rec{l}_{m}")
                nc.vector.reciprocal(rec[:], dsb[:])
                msgt = sbm.tile([128, 512], F32, tag="msg", bufs=2,
                                name=f"msg{l}_{m}")
                nc.vector.tensor_mul(
                    msgt[:].rearrange("p (h c) -> p h c", h=8),
                    acc[:].rearrange("p (h c) -> p h c", h=8),
                    rec[:, :, None].broadcast_to([128, 8, 64]))
                nc.vector.tensor_add(pre_sb[:, m, :], msgt[:], pre_sb[:, m, :])
                sq = sbm.tile([128, 512], F32, tag="sq", bufs=2,
                              name=f"sq{l}_{m}")
                nc.scalar.activation(sq[:], pre_sb[:, m, :], AF.Square)
                nc.tensor.matmul(stat_ps[0:1, :], lhsT=mask_sb[:, m:m + 1],
                                 rhs=pre_sb[:, m, :], start=(m == 0),
                                 stop=(m == NT - 1), skip_group_check=True)
                nc.tensor.matmul(stat_ps[32:33, :], lhsT=mask_sb[:, m:m + 1],
                                 rhs=sq[:], start=(m == 0),
                                 stop=(m == NT - 1), skip_group_check=True)
                ch0 += CH[m]
                ga0 += GB[m]

            # -- BN stats AllReduce
            statacc = sbs.tile([1, 1024], F32, tag="statacc", bufs=2,
                               name=f"statacc{l}")
            nc.vector.tensor_copy(out=statacc[0:1, 0:512], in_=stat_ps[0:1, :])
            nc.vector.tensor_copy(out=statacc[0:1, 512:1024],
                                  in_=stat_ps[32:33, :])
            arin = dram.tile([1, 1024], F32, tag="arin", name=f"arin{l}")
            arout_d = dram.tile([1, 1024], F32, tag="arout",
                                addr_space="Shared", name=f"arout{l}")
            nc.sync.dma_start(out=arin[:], in_=statacc[:])
            nc.gpsimd.collective_compute(
                "AllReduce", mybir.AluOpType.add, replica_groups=rg,
                ins=[arin[:].opt()], outs=[arout_d[:].opt()])
            aro = sbs.tile([1, 1024], F32, tag="aro", bufs=1, name=f"aro{l}")
            nc.sync.dma_start(out=aro[:], in_=arout_d[:])

            # A = gamma * rstd ; Bb = beta - mu * A   (rows: [A; Bb])
            mu = sbs.tile([1, 512], F32, tag="mu", bufs=1, name=f"mu{l}")
            nc.scalar.activation(mu[:], aro[0:1, 0:512], AF.Copy, scale=1.0 / N)
            ex2 = sbs.tile([1, 512], F32, tag="ex2", bufs=1, name=f"ex2{l}")
            nc.scalar.activation(ex2[:], aro[0:1, 512:1024], AF.Copy,
                                 scale=1.0 / N)
            var = sbs.tile([1, 512], F32, tag="var", bufs=1, name=f"var{l}")
            nc.vector.tensor_mul(var[:], mu[:], mu[:])
            nc.vector.tensor_sub(var[:], ex2[:], var[:])
            stdt = sbs.tile([1, 512], F32, tag="stdt", bufs=1, name=f"stdt{l}")
            nc.scalar.activation(stdt[:], var[:], AF.Sqrt, bias=BN_EPS)
            rstd = sbs.tile([1, 512], F32, tag="rstd", bufs=1, name=f"rstd{l}")
            nc.vector.reciprocal(rstd[:], stdt[:])
            ab = sbs.tile([2, 512], F32, tag="ab", bufs=1, name=f"ab{l}")
            nc.vector.tensor_mul(ab[0:1, :], gam_sb[0:1, :], rstd[:])
            tmB = sbs.tile([1, 512], F32, tag="tmB", bufs=1, name=f"tmB{l}")
            nc.vector.tensor_mul(tmB[:], mu[:], ab[0:1, :])
            bbrow = sbs.tile([1, 512], F32, tag="bbrow", bufs=1,
                             name=f"bbrow{l}")
            nc.vector.tensor_sub(bbrow[:], bet_sb[0:1, :], tmB[:])
            nc.sync.dma_start(out=ab[1:2, :], in_=bbrow[:])

            abT = sbs.tile([128, 4, 2], F32, tag="abT", name=f"abT{l}")
            for kc in range(4):
                tp = ps.tile([128, 2], F32, tag="den", bufs=1,
                             name=f"abtp{l}_{kc}")
                nc.tensor.transpose(tp[:], ab[:, kc * 128:(kc + 1) * 128],
                                    identf[0:2, 0:2])
                nc.vector.tensor_copy(out=abT[:, kc, :], in_=tp[:])

            # -- h_next = relu(A*pre + Bb) in feature-major
            h_nxt = sbh.tile([128, 4, NLOC], BF, tag="h", name=f"h{l + 1}")
            for m in range(NT):
                for kc in range(4):
                    tp1 = ps.tile([128, 128], F32, tag="qd", bufs=2,
                                  name=f"tp1{l}_{m}_{kc}")
                    nc.tensor.transpose(
                        tp1[:], pre_sb[:, m, kc * 128:(kc + 1) * 128],
                        identf[:])
                    nc.scalar.activation(
                        h_nxt[:, kc, m * 128:(m + 1) * 128], tp1[:], AF.Relu,
                        scale=abT[:, kc, 0:1], bias=abT[:, kc, 1:2])

            h_cur = h_nxt

        # pool for the last layer (earlier layers pooled inside the loop,
        # overlapped with the next layer's AllGather)
        emit_pool(L - 1, h_cur)

    return nc


def _host_shard(x, edge_index, batch):
    """Build all per-core host-side index/constant arrays with tight
    per-block chunk packing (counts maxed over cores for SPMD)."""
    batch = np.asarray(batch)
    src = np.asarray(edge_index[0])
    dst = np.asarray(edge_index[1])
    n = x.shape[0]

    node_start = np.searchsorted(batch, np.arange(0, B, GPC))
    node_end = np.searchsorted(batch, np.arange(GPC - 1, B, GPC), side="right")
    nloc = node_end - node_start
    NT = int(-(-nloc.max() // 128))
    NLOC = NT * 128

    core_of_node = batch // GPC
    local_of_node = np.arange(n) - node_start[core_of_node]
    grow_of_node = core_of_node * NLOC + local_of_node

    ec = core_of_node[dst]
    ld = local_of_node[dst]

    # per-(core,block) edge counts -> per-block chunk counts (max over cores)
    counts = np.zeros((NCORE, NT), np.int64)
    for c in range(NCORE):
        m = ec == c
        counts[c] = np.bincount(ld[m] // 128, minlength=NT)
    CH = [max(1, int(v)) for v in (-(-counts.max(axis=0) // 128))]
    GB = [int(-(-chm // 4)) for chm in CH]
    CHT = sum(CH)
    NGA = sum(GB)

    idx16 = np.full((NCORE, 128, NGA * (GI // 16)), -1, np.int16)
    stc = np.zeros((NCORE, 128, CHT * 128), np.float32)
    stt = np.zeros((NCORE, 128, CHT * 128), np.float32)
    mask = np.zeros((NCORE, 128, NT), np.float32)
    spool = np.zeros((NCORE, 128, NT * GPC), np.float32)
    cntr = np.zeros((NCORE, GPC, 1), np.float32)
    xT = np.zeros((NCORE, 128, NLOC), np.float32)

    jj = np.arange(128)
    x = np.asarray(x)
    for c in range(NCORE):
        ns, nl = node_start[c], nloc[c]
        xT[c, :, :nl] = x[ns:ns + nl].T
        m2 = np.zeros(NLOC, np.float32)
        m2[:nl] = 1.0
        mask[c] = m2.reshape(NT, 128).T
        gl = batch[ns:ns + nl] - c * GPC
        sp = np.zeros((NLOC, GPC), np.float32)
        sp[np.arange(nl), gl] = 1.0
        spool[c] = sp.reshape(NT, 128, GPC).transpose(1, 0, 2).reshape(
            128, NT * GPC)
        cnt = sp.sum(axis=0)
        cntr[c, :, 0] = 1.0 / np.maximum(cnt, 1.0)

        eids = np.nonzero(ec == c)[0]
        order = np.argsort(ld[eids], kind="stable")
        eids = eids[order]
        lds = ld[eids]
        srows = grow_of_node[src[eids]]
        blk = lds // 128
        bc = np.bincount(blk, minlength=NT)
        pos = 0
        ch0 = 0
        ga0 = 0
        for m in range(NT):
            n_ = int(bc[m])
            nslot = GB[m] * 512
            # pad slots gather row 0 (negative "skip" indices hang the
            # gather ucode on this runtime); dst -1 keeps the indicator
            # column zero so they contribute nothing
            a_src = np.zeros(nslot, np.int64)
            a_dst = np.full(nslot, -1.0, np.float32)
            a_src[:n_] = srows[pos:pos + n_]
            a_dst[:n_] = (lds[pos:pos + n_] % 128).astype(np.float32)
            pos += n_
            # gather indices: idx i of gather g -> partition i%16, col i//16
            w = a_src.reshape(GB[m], GI // 16, 16)
            wt = w.transpose(0, 2, 1).reshape(GB[m], 16, GI // 16)
            for g in range(GB[m]):
                cols = slice((ga0 + g) * (GI // 16), (ga0 + g + 1) * (GI // 16))
                for r in range(8):
                    idx16[c, r * 16:(r + 1) * 16, cols] = wt[g]
            # per-chunk indicator matrices, both orientations
            for ci in range(CH[m]):
                col = a_dst[ci * 128:(ci + 1) * 128]
                sl = slice((ch0 + ci) * 128, (ch0 + ci + 1) * 128)
                stc[c, :, sl] = (col[:, None] == jj[None, :]).astype(
                    np.float32)
                stt[c, :, sl] = (col[None, :] == jj[:, None]).astype(
                    np.float32)
            ch0 += CH[m]
            ga0 += GB[m]

    return (NT, CH, GB, node_start, idx16, stc, stt, mask, spool, cntr, xT)


def kernel(x, edge_index, batch, W0_q, b0_q, W0_k, b0_k, W0_v, b0_v,
           W0_s, b0_s, Wq, bq, Wk, bk, Wv, bv, Ws, bs, gamma, beta):
    from concourse.bass_utils import run_bass_kernel_spmd

    (NT, CH, GB, node_start, idx16, stc, stt, mask, spool, cntr, xT) = \
        _host_shard(x, edge_index, batch)

    W0a = np.concatenate([np.asarray(W0_q), np.asarray(W0_k),
                          np.asarray(W0_v), np.asarray(W0_s)], axis=1)
    WRa = np.zeros((7 * 2048, 512), np.float32)
    Wstack = [np.asarray(Wq), np.asarray(Wk), np.asarray(Wv), np.asarray(Ws)]
    for li in range(7):
        for pr in range(4):
            for kc in range(4):
                r0 = li * 2048 + pr * 512 + kc * 128
                WRa[r0:r0 + 128] = Wstack[pr][li][kc * 128:(kc + 1) * 128, :]
    BIASa = np.zeros((8, 2048), np.float32)
    BIASa[0] = np.concatenate([np.asarray(b0_q), np.asarray(b0_k),
                               np.asarray(b0_v), np.asarray(b0_s)])
    bstack = [np.asarray(bq), np.asarray(bk), np.asarray(bv), np.asarray(bs)]
    for li in range(7):
        BIASa[li + 1] = np.concatenate([bstack[pr][li] for pr in range(4)])

    iota_f = np.tile(np.arange(128, dtype=np.float32)[None, :], (128, 1))
    ones1 = np.ones((1, 128), np.float32)
    ident = np.eye(128, dtype=np.float32)

    common = {
        "W0": _to_bf(W0a), "WR": _to_bf(WRa),
        "BIAS": _to_bf(BIASa.reshape(1, -1)),
        "GAM": np.asarray(gamma, np.float32).reshape(1, -1),
        "BET": np.asarray(beta, np.float32).reshape(1, -1),
        "ONES1": _to_bf(ones1), "IDENTF": ident, "IDENTB": _to_bf(ident),
    }
    in_maps = []
    for c in range(NCORE):
        in_maps.append(dict(
            common,
            XT=_to_bf(xT[c]), IDX=idx16[c],
            STC=_to_bf(stc[c]), STT=_to_bf(stt[c]),
            MASK=mask[c], SPOOL=_to_bf(spool[c]), CNTR=cntr[c],
        ))

    nc = _build_nc(NT, CH, GB)
    nc.compile()
    res = run_bass_kernel_spmd(nc, in_maps, list(range(NCORE)))
    out = np.zeros((B, L * 512), np.float32)
    for c in range(NCORE):
        out[c * GPC:(c + 1) * GPC] = res.results[c]["POOLED"]
    return out


if __name__ == "__main__":
    pass


# revision 54
# speedup vs baseline: 1.8543x; 1.8543x over previous
"""GraphTransformerEncoder (8-layer TransformerConv + BN + ReLU + mean-pool)
on 8 Trainium2 NeuronCores via Bass/Tile.

Sharding: graph-parallel. Core c owns graphs [8c, 8c+8) -> a contiguous node
range (batch is sorted). Edges are owned by the core of their dst node, sorted
by dst, and packed into per-128-node-block chunk lists with per-block counts
fitted to the data (max over cores, so the single SPMD program works for all).
Per layer each core computes K/V projections, AllGathers the K|V table (bf16),
computes Q/root projections while the collective runs, DMA-gathers K|V rows
for its edges' src nodes, computes the edge softmax via segment-indicator
matmuls (indicator transposed table host-precomputed and SBUF-resident), and
applies BN (global stats via a tiny AllReduce) + ReLU, then mean-pools.
"""

import numpy as np
import ml_dtypes

import concourse.bass as bass
import concourse.bacc as bacc
import concourse.mybir as mybir
import concourse.tile as tile
from concourse import library_config
from contextlib import ExitStack

BF = mybir.dt.bfloat16
F8 = mybir.dt.float8e4
F32 = mybir.dt.float32
I16 = mybir.dt.int16
AF = mybir.ActivationFunctionType

# problem constants
N, E, F, H, C, L, B = 10000, 160000, 128, 8, 64, 8, 64
D = H * C  # 512
BN_EPS = 1e-5

NCORE = 8
GPC = B // NCORE        # graphs per core = 8
GI = 512                # indices per dma_gather (4 chunks)
GBUFS = 5               # gather tiles in flight
RW = 1536               # K|V row bytes: 1KB bf16 K + 512B fp8 V


def _to_bf(a):
    return np.asarray(a, dtype=np.float32).astype(ml_dtypes.bfloat16)


def _build_nc(NT, CH, GB):
    """Build the SPMD program. NT: node blocks per core; CH[m]: chunks per
    block (128 edge slots each); GB[m]: gathers per block (512 slots each)."""
    NLOC = NT * 128
    KVROWS = NCORE * NLOC
    CHT = sum(CH)
    NGA = sum(GB)

    nc = bacc.Bacc("TRN2", num_devices=NCORE,
                  target_bir_lowering=False, debug=False)
    rg = [list(range(NCORE))]

    # ---- I/O -----------------------------------------------------------
    XT = nc.dram_tensor("XT", [128, NLOC], BF, kind="ExternalInput")
    W0 = nc.dram_tensor("W0", [128, 4 * 512], BF, kind="ExternalInput")
    WR = nc.dram_tensor("WR", [7 * 2048, 512], BF, kind="ExternalInput")
    BIAS = nc.dram_tensor("BIAS", [1, 8 * 2048], BF, kind="ExternalInput")
    GAM = nc.dram_tensor("GAM", [1, 8 * 512], F32, kind="ExternalInput")
    BET = nc.dram_tensor("BET", [1, 8 * 512], F32, kind="ExternalInput")
    IDX = nc.dram_tensor("IDX", [128, NGA * (GI // 16)], I16, kind="ExternalInput")
    STC = nc.dram_tensor("STC", [128, CHT * 128], BF, kind="ExternalInput")
    STT = nc.dram_tensor("STT", [128, CHT * 128], BF, kind="ExternalInput")
    ONES1 = nc.dram_tensor("ONES1", [1, 128], BF, kind="ExternalInput")
    IDENTF = nc.dram_tensor("IDENTF", [128, 128], F32, kind="ExternalInput")
    IDENTB = nc.dram_tensor("IDENTB", [128, 128], BF, kind="ExternalInput")
    MASK = nc.dram_tensor("MASK", [128, NT], F32, kind="ExternalInput")
    SPOOL = nc.dram_tensor("SPOOL", [128, NT * GPC], BF, kind="ExternalInput")
    CNTR = nc.dram_tensor("CNTR", [GPC, 1], F32, kind="ExternalInput")
    OUT = nc.dram_tensor("POOLED", [GPC, L * 512], F32, kind="ExternalOutput")

    with tile.TileContext(nc) as tc, ExitStack() as ctx:
        sb1 = ctx.enter_context(tc.tile_pool(name="sb1", bufs=1))
        sbh = ctx.enter_context(tc.tile_pool(name="sbh", bufs=2))
        sbw = ctx.enter_context(tc.tile_pool(name="sbw", bufs=2))
        sbs = ctx.enter_context(tc.tile_pool(name="sbs", bufs=3))
        sbg = ctx.enter_context(tc.tile_pool(name="sbg", bufs=GBUFS))
        sbm = ctx.enter_context(tc.tile_pool(name="sbm", bufs=2))
        ps = ctx.enter_context(tc.tile_pool(name="ps", bufs=1, space="PSUM"))
        dram = ctx.enter_context(tc.tile_pool(name="dram", bufs=2, space="DRAM"))

        def load1(src, shape, dtype, name):
            t = sb1.tile(shape, dtype, name=name)
            nc.sync.dma_start(out=t[:], in_=src[:])
            return t

        ones1 = load1(ONES1, [1, 128], BF, "ones1")
        identf = load1(IDENTF, [128, 128], F32, "identf")
        identb = load1(IDENTB, [128, 128], BF, "identb")
        idx_sb = load1(IDX, [128, NGA * (GI // 16)], I16, "idx_sb")
        mask_sb = load1(MASK, [128, NT], F32, "mask_sb")
        spool_sb = load1(SPOOL, [128, NT * GPC], BF, "spool_sb")
        cntr_sb = load1(CNTR, [GPC, 1], F32, "cntr_sb")

        czero = sb1.tile([128, 1], F32, name="czero")
        nc.vector.memset(czero[:], 0.0)
        ceps = sb1.tile([128, 1], F32, name="ceps")
        nc.vector.memset(ceps[:], BN_EPS)
        nc.const_aps.aps[(F32, 0.0)] = czero[:]
        nc.const_aps.aps[(F32, BN_EPS)] = ceps[:]

        nc.gpsimd.load_library(library_config.mlp)

        h_cur = sbh.tile([128, 4, NLOC], BF, tag="h", name="h0")
        nc.sync.dma_start(out=h_cur[:, 0, :], in_=XT[:, :])

        # NaN guard: first-layer gathers skip -1 slots, leaving stale SBUF.
        for i in range(GBUFS):
            gz = sbg.tile([128, 4, RW], F8, tag="g", name=f"gz{i}")
            nc.vector.memset(gz[:], 0.0)

        def emit_pool(lp, h):
            """Mean-pool layer lp's output h (feature-major) into OUT."""
            poolp = ps.tile([8, 512], F32, tag="stat", bufs=1,
                            name=f"poolp{lp}")
            for m in range(NT):
                hnm = sbm.tile([128, 512], BF, tag="hnm", bufs=2,
                               name=f"hnm{lp}_{m}")
                for kc in range(4):
                    tp2 = ps.tile([128, 128], BF, tag="den", bufs=1,
                                  name=f"tp2{lp}_{m}_{kc}")
                    nc.tensor.transpose(
                        tp2[:], h[:, kc, m * 128:(m + 1) * 128], identb[:])
                    nc.scalar.activation(hnm[:, kc * 128:(kc + 1) * 128],
                                         tp2[:], AF.Copy)
                nc.tensor.matmul(poolp[:],
                                 lhsT=spool_sb[:, m * GPC:(m + 1) * GPC],
                                 rhs=hnm[:], start=(m == 0),
                                 stop=(m == NT - 1))
            pool_sb = sbs.tile([GPC, 512], F32, tag="poolsb", bufs=2,
                               name=f"pool{lp}")
            nc.scalar.activation(pool_sb[:], poolp[:], AF.Identity,
                                 scale=cntr_sb[:, 0:1])
            nc.sync.dma_start(out=OUT[:, lp * 512:(lp + 1) * 512],
                              in_=pool_sb[:])

        for l in range(L):
            KIN = 1 if l == 0 else 4

            w_sb = sbw.tile([128, 4 * KIN, 512], BF, tag="w", name=f"w{l}")
            if l == 0:
                nc.sync.dma_start(
                    out=w_sb[:], in_=W0[:, :].rearrange("p (c n) -> p c n", c=4))
            else:
                nc.sync.dma_start(
                    out=w_sb[:],
                    in_=WR[(l - 1) * 2048: l * 2048, :].rearrange(
                        "(c p) n -> p c n", p=128))

            bias_sb = sbs.tile([1, 2048], BF, tag="bias", bufs=2,
                               name=f"bias{l}")
            nc.sync.dma_start(out=bias_sb[:],
                              in_=BIAS[0:1, l * 2048:(l + 1) * 2048])
            gam_sb = sbs.tile([1, 512], F32, tag="gam", bufs=2, name=f"gam{l}")
            nc.sync.dma_start(out=gam_sb[:], in_=GAM[0:1, l * 512:(l + 1) * 512])
            bet_sb = sbs.tile([1, 512], F32, tag="bet", bufs=2, name=f"bet{l}")
            nc.sync.dma_start(out=bet_sb[:], in_=BET[0:1, l * 512:(l + 1) * 512])

            kv_loc = dram.tile([NLOC, RW], F8, tag="kvloc", name=f"kvloc{l}")
            kv_full = dram.tile([KVROWS, RW], F8, tag="kvfull",
                                addr_space="Shared", name=f"kvfull{l}")

            # -- phase A1: K,V projections -> kv_loc; AllGather in two halves
            # so the first collective overlaps the second half's projections
            for m in range(NT):
                kv_sb = sbm.tile([128, RW], F8, tag="kvp", bufs=2,
                                 name=f"kv{l}_{m}")
                for pr in (1, 2):  # 1=k 2=v
                    pp = ps.tile([128, 2, 512], F32, tag="qd", bufs=2,
                                 name=f"pp{l}_{m}_{pr}")
                    for kc in range(KIN):
                        nc.tensor.matmul(
                            pp[:, 0, :], lhsT=h_cur[:, kc, m * 128:(m + 1) * 128],
                            rhs=w_sb[:, pr * KIN + kc, :],
                            start=(kc == 0), stop=False)
                    nc.tensor.matmul(
                        pp[:, 0, :], lhsT=ones1[:],
                        rhs=bias_sb[0:1, pr * 512:(pr + 1) * 512],
                        start=False, stop=True)
                    if pr == 1:   # K half, bf16
                        nc.scalar.activation(
                            kv_sb[:, 0:1024].bitcast(BF), pp[:, 0, :], AF.Copy)
                    else:         # V half, fp8e4
                        nc.scalar.activation(
                            kv_sb[:, 1024:RW], pp[:, 0, :], AF.Copy)
                nc.sync.dma_start(out=kv_loc[m * 128:(m + 1) * 128, :],
                                  in_=kv_sb[:])

            nc.gpsimd.collective_compute(
                "AllGather", mybir.AluOpType.bypass, replica_groups=rg,
                ins=[kv_loc[:].opt()], outs=[kv_full[:].opt()])

            # previous layer's pooling, deferred into the AllGather window
            if l > 0:
                emit_pool(l - 1, h_cur)

            # -- phase A2 (overlaps AllGather): Q + root projections
            Q_sb = sbm.tile([128, NT, 512], BF, tag="q", bufs=1, name=f"q{l}")
            pre_sb = sbm.tile([128, NT, 512], F32, tag="pre", bufs=1,
                              name=f"pre{l}")
            for m in range(NT):
                for pr in (0, 3):  # 0=q 3=root
                    pp = ps.tile([128, 2, 512], F32, tag="qd", bufs=2,
                                 name=f"qr{l}_{m}_{pr}")
                    for kc in range(KIN):
                        nc.tensor.matmul(
                            pp[:, 0, :], lhsT=h_cur[:, kc, m * 128:(m + 1) * 128],
                            rhs=w_sb[:, pr * KIN + kc, :],
                            start=(kc == 0), stop=False)
                    nc.tensor.matmul(
                        pp[:, 0, :], lhsT=ones1[:],
                        rhs=bias_sb[0:1, pr * 512:(pr + 1) * 512],
                        start=False, stop=True)
                    if pr == 0:
                        nc.scalar.activation(Q_sb[:, m, :], pp[:, 0, :], AF.Copy)
                    else:
                        nc.scalar.activation(pre_sb[:, m, :], pp[:, 0, :],
                                             AF.Copy)

            # -- phase B: edge stage
            # rows 0 / 32: sum / sum-of-squares (matmul out base partition
            # must be 0, 32, or 64)
            stat_ps = ps.tile([33, 512], F32, tag="stat", bufs=1,
                              name=f"stat{l}")
            ch0 = 0
            ga0 = 0
            for m in range(NT):
                # stream both indicator orientations for this block (tiny,
                # on the otherwise-idle regular DMA queues)
                stb = sbs.tile([128, CH[m] * 128], BF, tag="stb", bufs=2,
                               name=f"stb{l}_{m}")
                nc.sync.dma_start(
                    out=stb[:], in_=STC[:, ch0 * 128:(ch0 + CH[m]) * 128])
                sttb = sbs.tile([128, CH[m] * 128], BF, tag="sttb", bufs=2,
                                name=f"sttb{l}_{m}")
                nc.sync.dma_start(
                    out=sttb[:], in_=STT[:, ch0 * 128:(ch0 + CH[m]) * 128])
                acc = ps.tile([128, 512], F32, tag="acc", bufs=2,
                              name=f"acc{l}_{m}")
                den = ps.tile([128, 8], F32, tag="den", bufs=1,
                              name=f"den{l}_{m}")
                for g in range(GB[m]):
                    nch = min(4, CH[m] - 4 * g)
                    gt = sbg.tile([128, 4, RW], F8, tag="g",
                                  name=f"gt{l}_{m}_{g}")
                    ga = ga0 + g
                    nidx = nch * 128  # partial tail gathers move fewer rows
                    nc.gpsimd.dma_gather(
                        gt[:, 0:nch, :], kv_full[:, :],
                        idx_sb[:, ga * (GI // 16):
                               ga * (GI // 16) + nidx // 16],
                        nidx, nidx, RW)
                    lg = sbs.tile([128, 4, 8], F32, tag="lg", bufs=4,
                                  name=f"lg{l}_{ga}")
                    pbf = sbs.tile([128, 4, 8], BF, tag="p", bufs=4,
                                   name=f"p{l}_{ga}")
                    pv = sbm.tile([128, 4, 512], BF, tag="pv", bufs=2,
                                  name=f"pv{l}_{ga}")
                    for g2 in range((nch + 1) // 2):
                        n2 = min(2, nch - 2 * g2)
                        qd = ps.tile([128, 2, 512], F32, tag="qd", bufs=2,
                                     name=f"qd{l}_{ga}_{g2}")
                        for i in range(n2):
                            ci = 4 * g + 2 * g2 + i
                            nc.tensor.matmul(
                                qd[:, i, :],
                                lhsT=sttb[:, ci * 128:(ci + 1) * 128],
                                rhs=Q_sb[:, m, :], start=True, stop=True)
                        nc.scalar.activation(pv[:, 2 * g2:2 * g2 + n2, :],
                                             qd[:, 0:n2, :], AF.Copy)
                    pvv = pv[:, 0:nch, :]
                    nc.vector.tensor_mul(pvv, pvv,
                                         gt[:, 0:nch, 0:1024].bitcast(BF))
                    nc.vector.tensor_reduce(
                        lg[:, 0:nch, :],
                        pvv.rearrange("p n (h c) -> p n h c", h=8),
                        mybir.AxisListType.X, mybir.AluOpType.add)
                    nc.scalar.activation(pbf[:, 0:nch, :], lg[:, 0:nch, :],
                                         AF.Exp, scale=0.125)
                    nc.vector.tensor_mul(
                        pv[:, 0:nch, :].rearrange("p n (h c) -> p n h c", h=8),
                        gt[:, 0:nch, 1024:RW].rearrange(
                            "p n (h c) -> p n h c", h=8),
                        pbf[:, 0:nch, :, None].broadcast_to([128, nch, 8, 64]))
                    for cc in range(nch):
                        ci = 4 * g + cc
                        first = (ci == 0)
                        last = (ci == CH[m] - 1)
                        nc.tensor.matmul(acc[:],
                                         lhsT=stb[:, ci * 128:(ci + 1) * 128],
                                         rhs=pv[:, cc, :],
                                         start=first, stop=last)
                        nc.tensor.matmul(den[:],
                                         lhsT=stb[:, ci * 128:(ci + 1) * 128],
                                         rhs=pbf[:, cc, :],
                                         start=first, stop=last)

                # block finalize: normalize, add root (staged in pre_sb), stats
                dsb = sbs.tile([128, 8], F32, tag="dsb", name=f"dsb{l}_{m}")
                nc.scalar.activation(dsb[:], den[:], AF.Copy, bias=1e-16)
                rec = sbs.tile([128, 8], F32, tag="rec", name=f"rec{l}_{m}")
                nc.vector.reciprocal(rec[:], dsb[:])
                msgt = sbm.tile([128, 512], F32, tag="msg", bufs=2,
                                name=f"msg{l}_{m}")
                nc.vector.tensor_mul(
                    msgt[:].rearrange("p (h c) -> p h c", h=8),
                    acc[:].rearrange("p (h c) -> p h c", h=8),
                    rec[:, :, None].broadcast_to([128, 8, 64]))
                nc.vector.tensor_add(pre_sb[:, m, :], msgt[:], pre_sb[:, m, :])
                sq = sbm.tile([128, 512], F32, tag="sq", bufs=2,
                              name=f"sq{l}_{m}")
                nc.scalar.activation(sq[:], pre_sb[:, m, :], AF.Square)
                nc.tensor.matmul(stat_ps[0:1, :], lhsT=mask_sb[:, m:m + 1],
                                 rhs=pre_sb[:, m, :], start=(m == 0),
                                 stop=(m == NT - 1), skip_group_check=True)
                nc.tensor.matmul(stat_ps[32:33, :], lhsT=mask_sb[:, m:m + 1],
                                 rhs=sq[:], start=(m == 0),
                                 stop=(m == NT - 1), skip_group_check=True)
                ch0 += CH[m]
                ga0 += GB[m]

            # -- BN stats AllReduce
            statacc = sbs.tile([1, 1024], F32, tag="statacc", bufs=2,
                               name=f"statacc{l}")
            nc.vector.tensor_copy(out=statacc[0:1, 0:512], in_=stat_ps[0:1, :])
            nc.vector.tensor_copy(out=statacc[0:1, 512:1024],
                                  in_=stat_ps[32:33, :])
            arin = dram.tile([1, 1024], F32, tag="arin", name=f"arin{l}")
            arout_d = dram.tile([1, 1024], F32, tag="arout",
                                addr_space="Shared", name=f"arout{l}")
            nc.sync.dma_start(out=arin[:], in_=statacc[:])
            nc.gpsimd.collective_compute(
                "AllReduce", mybir.AluOpType.add, replica_groups=rg,
                ins=[arin[:].opt()], outs=[arout_d[:].opt()])
            aro = sbs.tile([1, 1024], F32, tag="aro", bufs=1, name=f"aro{l}")
            nc.sync.dma_start(out=aro[:], in_=arout_d[:])

            # A = gamma * rstd ; Bb = beta - mu * A   (rows: [A; Bb])
            mu = sbs.tile([1, 512], F32, tag="mu", bufs=1, name=f"mu{l}")
            nc.scalar.activation(mu[:], aro[0:1, 0:512], AF.Copy, scale=1.0 / N)
            ex2 = sbs.tile([1, 512], F32, tag="ex2", bufs=1, name=f"ex2{l}")
            nc.scalar.activation(ex2[:], aro[0:1, 512:1024], AF.Copy,
                                 scale=1.0 / N)
            var = sbs.tile([1, 512], F32, tag="var", bufs=1, name=f"var{l}")
            nc.vector.tensor_mul(var[:], mu[:], mu[:])
            nc.vector.tensor_sub(var[:], ex2[:], var[:])
            stdt = sbs.tile([1, 512], F32, tag="stdt", bufs=1, name=f"stdt{l}")
            nc.scalar.activation(stdt[:], var[:], AF.Sqrt, bias=BN_EPS)
            rstd = sbs.tile([1, 512], F32, tag="rstd", bufs=1, name=f"rstd{l}")
            nc.vector.reciprocal(rstd[:], stdt[:])
            ab = sbs.tile([2, 512], F32, tag="ab", bufs=1, name=f"ab{l}")
            nc.vector.tensor_mul(ab[0:1, :], gam_sb[0:1, :], rstd[:])
            tmB = sbs.tile([1, 512], F32, tag="tmB", bufs=1, name=f"tmB{l}")
            nc.vector.tensor_mul(tmB[:], mu[:], ab[0:1, :])
            bbrow = sbs.tile([1, 512], F32, tag="bbrow", bufs=1,
                             name=f"bbrow{l}")
            nc.vector.tensor_sub(bbrow[:], bet_sb[0:1, :], tmB[:])
            nc.sync.dma_start(out=ab[1:2, :], in_=bbrow[:])

            abT = sbs.tile([128, 4, 2], F32, tag="abT", name=f"abT{l}")
            for kc in range(4):
                tp = ps.tile([128, 2], F32, tag="den", bufs=1,
                             name=f"abtp{l}_{kc}")
                nc.tensor.transpose(tp[:], ab[:, kc * 128:(kc + 1) * 128],
                                    identf[0:2, 0:2])
                nc.vector.tensor_copy(out=abT[:, kc, :], in_=tp[:])

            # -- h_next = relu(A*pre + Bb) in feature-major
            h_nxt = sbh.tile([128, 4, NLOC], BF, tag="h", name=f"h{l + 1}")
            for m in range(NT):
                for kc in range(4):
                    tp1 = ps.tile([128, 128], F32, tag="qd", bufs=2,
                                  name=f"tp1{l}_{m}_{kc}")
                    nc.tensor.transpose(
                        tp1[:], pre_sb[:, m, kc * 128:(kc + 1) * 128],
                        identf[:])
                    nc.scalar.activation(
                        h_nxt[:, kc, m * 128:(m + 1) * 128], tp1[:], AF.Relu,
                        scale=abT[:, kc, 0:1], bias=abT[:, kc, 1:2])

            h_cur = h_nxt

        # pool for the last layer (earlier layers pooled inside the loop,
        # overlapped with the next layer's AllGather)
        emit_pool(L - 1, h_cur)

    return nc


def _host_shard(x, edge_index, batch):
    """Build all per-core host-side index/constant arrays with tight
    per-block chunk packing (counts maxed over cores for SPMD)."""
    batch = np.asarray(batch)
    src = np.asarray(edge_index[0])
    dst = np.asarray(edge_index[1])
    n = x.shape[0]

    node_start = np.searchsorted(batch, np.arange(0, B, GPC))
    node_end = np.searchsorted(batch, np.arange(GPC - 1, B, GPC), side="right")
    nloc = node_end - node_start
    NT = int(-(-nloc.max() // 128))
    NLOC = NT * 128

    core_of_node = batch // GPC
    local_of_node = np.arange(n) - node_start[core_of_node]
    grow_of_node = core_of_node * NLOC + local_of_node

    ec = core_of_node[dst]
    ld = local_of_node[dst]

    # per-(core,block) edge counts -> per-block chunk counts (max over cores)
    counts = np.zeros((NCORE, NT), np.int64)
    for c in range(NCORE):
        m = ec == c
        counts[c] = np.bincount(ld[m] // 128, minlength=NT)
    CH = [max(1, int(v)) for v in (-(-counts.max(axis=0) // 128))]
    GB = [int(-(-chm // 4)) for chm in CH]
    CHT = sum(CH)
    NGA = sum(GB)

    idx16 = np.full((NCORE, 128, NGA * (GI // 16)), -1, np.int16)
    stc = np.zeros((NCORE, 128, CHT * 128), np.float32)
    stt = np.zeros((NCORE, 128, CHT * 128), np.float32)
    mask = np.zeros((NCORE, 128, NT), np.float32)
    spool = np.zeros((NCORE, 128, NT * GPC), np.float32)
    cntr = np.zeros((NCORE, GPC, 1), np.float32)
    xT = np.zeros((NCORE, 128, NLOC), np.float32)

    jj = np.arange(128)
    x = np.asarray(x)
    for c in range(NCORE):
        ns, nl = node_start[c], nloc[c]
        xT[c, :, :nl] = x[ns:ns + nl].T
        m2 = np.zeros(NLOC, np.float32)
        m2[:nl] = 1.0
        mask[c] = m2.reshape(NT, 128).T
        gl = batch[ns:ns + nl] - c * GPC
        sp = np.zeros((NLOC, GPC), np.float32)
        sp[np.arange(nl), gl] = 1.0
        spool[c] = sp.reshape(NT, 128, GPC).transpose(1, 0, 2).reshape(
            128, NT * GPC)
        cnt = sp.sum(axis=0)
        cntr[c, :, 0] = 1.0 / np.maximum(cnt, 1.0)

        eids = np.nonzero(ec == c)[0]
        order = np.argsort(ld[eids], kind="stable")
        eids = eids[order]
        lds = ld[eids]
        srows = grow_of_node[src[eids]]
        blk = lds // 128
        bc = np.bincount(blk, minlength=NT)
        pos = 0
        ch0 = 0
        ga0 = 0
        for m in range(NT):
            n_ = int(bc[m])
            nslot = GB[m] * 512
            # pad slots gather row 0 (negative "skip" indices hang the
            # gather ucode on this runtime); dst -1 keeps the indicator
            # column zero so they contribute nothing
            a_src = np.zeros(nslot, np.int64)
            a_dst = np.full(nslot, -1.0, np.float32)
            a_src[:n_] = srows[pos:pos + n_]
            a_dst[:n_] = (lds[pos:pos + n_] % 128).astype(np.float32)
            pos += n_
            # gather indices: idx i of gather g -> partition i%16, col i//16
            w = a_src.reshape(GB[m], GI // 16, 16)
            wt = w.transpose(0, 2, 1).reshape(GB[m], 16, GI // 16)
            for g in range(GB[m]):
                cols = slice((ga0 + g) * (GI // 16), (ga0 + g + 1) * (GI // 16))
                for r in range(8):
                    idx16[c, r * 16:(r + 1) * 16, cols] = wt[g]
            # per-chunk indicator matrices, both orientations
            for ci in range(CH[m]):
                col = a_dst[ci * 128:(ci + 1) * 128]
                sl = slice((ch0 + ci) * 128, (ch0 + ci + 1) * 128)
                stc[c, :, sl] = (col[:, None] == jj[None, :]).astype(
                    np.float32)
                stt[c, :, sl] = (col[None, :] == jj[:, None]).astype(
                    np.float32)
            ch0 += CH[m]
            ga0 += GB[m]

    return (NT, CH, GB, node_start, idx16, stc, stt, mask, spool, cntr, xT)


def kernel(x, edge_index, batch, W0_q, b0_q, W0_k, b0_k, W0_v, b0_v,
           W0_s, b0_s, Wq, bq, Wk, bk, Wv, bv, Ws, bs, gamma, beta):
    from concourse.bass_utils import run_bass_kernel_spmd

    (NT, CH, GB, node_start, idx16, stc, stt, mask, spool, cntr, xT) = \
        _host_shard(x, edge_index, batch)

    W0a = np.concatenate([np.asarray(W0_q), np.asarray(W0_k),
                          np.asarray(W0_v), np.asarray(W0_s)], axis=1)
    WRa = np.zeros((7 * 2048, 512), np.float32)
    Wstack = [np.asarray(Wq), np.asarray(Wk), np.asarray(Wv), np.asarray(Ws)]
    for li in range(7):
        for pr in range(4):
            for kc in range(4):
                r0 = li * 2048 + pr * 512 + kc * 128
                WRa[r0:r0 + 128] = Wstack[pr][li][kc * 128:(kc + 1) * 128, :]
    BIASa = np.zeros((8, 2048), np.float32)
    BIASa[0] = np.concatenate([np.asarray(b0_q), np.asarray(b0_k),
                               np.asarray(b0_v), np.asarray(b0_s)])
    bstack = [np.asarray(bq), np.asarray(bk), np.asarray(bv), np.asarray(bs)]
    for li in range(7):
        BIASa[li + 1] = np.concatenate([bstack[pr][li] for pr in range(4)])

    iota_f = np.tile(np.arange(128, dtype=np.float32)[None, :], (128, 1))
    ones1 = np.ones((1, 128), np.float32)
    ident = np.eye(128, dtype=np.float32)

    common = {
        "W0": _to_bf(W0a), "WR": _to_bf(WRa),
        "BIAS": _to_bf(BIASa.reshape(1, -1)),
        "GAM": np.asarray(gamma, np.float32).reshape(1, -1),
        "BET": np.asarray(beta, np.float32).reshape(1, -1),
        "ONES1": _to_bf(ones1), "IDENTF": ident, "IDENTB": _to_bf(ident),
    }
    in_maps = []
    for c in range(NCORE):
        in_maps.append(dict(
            common,
            XT=_to_bf(xT[c]), IDX=idx16[c],
            STC=_to_bf(stc[c]), STT=_to_bf(stt[c]),
            MASK=mask[c], SPOOL=_to_bf(spool[c]), CNTR=cntr[c],
        ))

    nc = _build_nc(NT, CH, GB)
    nc.compile()
    res = run_bass_kernel_spmd(nc, in_maps, list(range(NCORE)))
    out = np.zeros((B, L * 512), np.float32)
    for c in range(NCORE):
        out[c * GPC:(c + 1) * GPC] = res.results[c]["POOLED"]
    return out


if __name__ == "__main__":
    pass


# revision 56
# speedup vs baseline: 1.8651x; 1.0059x over previous
"""GraphTransformerEncoder (8-layer TransformerConv + BN + ReLU + mean-pool)
on 8 Trainium2 NeuronCores via Bass/Tile.

Sharding: graph-parallel. Core c owns graphs [8c, 8c+8) -> a contiguous node
range (batch is sorted). Edges are owned by the core of their dst node, sorted
by dst, and packed into per-128-node-block chunk lists with per-block counts
fitted to the data (max over cores, so the single SPMD program works for all).
Per layer each core computes K/V projections, AllGathers the K|V table (bf16),
computes Q/root projections while the collective runs, DMA-gathers K|V rows
for its edges' src nodes, computes the edge softmax via segment-indicator
matmuls (indicator transposed table host-precomputed and SBUF-resident), and
applies BN (global stats via a tiny AllReduce) + ReLU, then mean-pools.
"""

import numpy as np
import ml_dtypes

import concourse.bass as bass
import concourse.bacc as bacc
import concourse.mybir as mybir
import concourse.tile as tile
from concourse import library_config
from contextlib import ExitStack

BF = mybir.dt.bfloat16
F8 = mybir.dt.float8e4
F32 = mybir.dt.float32
I16 = mybir.dt.int16
AF = mybir.ActivationFunctionType

# problem constants
N, E, F, H, C, L, B = 10000, 160000, 128, 8, 64, 8, 64
D = H * C  # 512
BN_EPS = 1e-5

NCORE = 8
GPC = B // NCORE        # graphs per core = 8
GI = 512                # indices per dma_gather (4 chunks)
GBUFS = 6               # gather tiles in flight
RW = 1536               # K|V row bytes: 1KB bf16 K + 512B fp8 V


def _to_bf(a):
    return np.asarray(a, dtype=np.float32).astype(ml_dtypes.bfloat16)


def _build_nc(NT, CH, GB):
    """Build the SPMD program. NT: node blocks per core; CH[m]: chunks per
    block (128 edge slots each); GB[m]: gathers per block (512 slots each)."""
    NLOC = NT * 128
    KVROWS = NCORE * NLOC
    CHT = sum(CH)
    NGA = sum(GB)

    nc = bacc.Bacc("TRN2", num_devices=NCORE,
                  target_bir_lowering=False, debug=False)
    rg = [list(range(NCORE))]

    # ---- I/O -----------------------------------------------------------
    XT = nc.dram_tensor("XT", [128, NLOC], BF, kind="ExternalInput")
    W0 = nc.dram_tensor("W0", [128, 4 * 512], BF, kind="ExternalInput")
    WR = nc.dram_tensor("WR", [7 * 2048, 512], BF, kind="ExternalInput")
    BIAS = nc.dram_tensor("BIAS", [1, 8 * 2048], BF, kind="ExternalInput")
    GAM = nc.dram_tensor("GAM", [1, 8 * 512], F32, kind="ExternalInput")
    BET = nc.dram_tensor("BET", [1, 8 * 512], F32, kind="ExternalInput")
    IDX = nc.dram_tensor("IDX", [128, NGA * (GI // 16)], I16, kind="ExternalInput")
    STC = nc.dram_tensor("STC", [128, CHT * 128], BF, kind="ExternalInput")
    STT = nc.dram_tensor("STT", [128, CHT * 128], BF, kind="ExternalInput")
    ONES1 = nc.dram_tensor("ONES1", [1, 128], BF, kind="ExternalInput")
    IDENTF = nc.dram_tensor("IDENTF", [128, 128], F32, kind="ExternalInput")
    IDENTB = nc.dram_tensor("IDENTB", [128, 128], BF, kind="ExternalInput")
    MASK = nc.dram_tensor("MASK", [128, NT], F32, kind="ExternalInput")
    SPOOL = nc.dram_tensor("SPOOL", [128, NT * GPC], BF, kind="ExternalInput")
    CNTR = nc.dram_tensor("CNTR", [GPC, 1], F32, kind="ExternalInput")
    OUT = nc.dram_tensor("POOLED", [GPC, L * 512], F32, kind="ExternalOutput")

    with tile.TileContext(nc) as tc, ExitStack() as ctx:
        sb1 = ctx.enter_context(tc.tile_pool(name="sb1", bufs=1))
        sbh = ctx.enter_context(tc.tile_pool(name="sbh", bufs=2))
        sbw = ctx.enter_context(tc.tile_pool(name="sbw", bufs=2))
        sbs = ctx.enter_context(tc.tile_pool(name="sbs", bufs=3))
        sbg = ctx.enter_context(tc.tile_pool(name="sbg", bufs=GBUFS))
        sbm = ctx.enter_context(tc.tile_pool(name="sbm", bufs=2))
        ps = ctx.enter_context(tc.tile_pool(name="ps", bufs=1, space="PSUM"))
        dram = ctx.enter_context(tc.tile_pool(name="dram", bufs=2, space="DRAM"))

        def load1(src, shape, dtype, name):
            t = sb1.tile(shape, dtype, name=name)
            nc.sync.dma_start(out=t[:], in_=src[:])
            return t

        ones1 = load1(ONES1, [1, 128], BF, "ones1")
        identf = load1(IDENTF, [128, 128], F32, "identf")
        identb = load1(IDENTB, [128, 128], BF, "identb")
        idx_sb = load1(IDX, [128, NGA * (GI // 16)], I16, "idx_sb")
        mask_sb = load1(MASK, [128, NT], F32, "mask_sb")
        spool_sb = load1(SPOOL, [128, NT * GPC], BF, "spool_sb")
        cntr_sb = load1(CNTR, [GPC, 1], F32, "cntr_sb")

        czero = sb1.tile([128, 1], F32, name="czero")
        nc.vector.memset(czero[:], 0.0)
        ceps = sb1.tile([128, 1], F32, name="ceps")
        nc.vector.memset(ceps[:], BN_EPS)
        nc.const_aps.aps[(F32, 0.0)] = czero[:]
        nc.const_aps.aps[(F32, BN_EPS)] = ceps[:]

        nc.gpsimd.load_library(library_config.mlp)

        h_cur = sbh.tile([128, 4, NLOC], BF, tag="h", name="h0")
        nc.sync.dma_start(out=h_cur[:, 0, :], in_=XT[:, :])

        # NaN guard: first-layer gathers skip -1 slots, leaving stale SBUF.
        for i in range(GBUFS):
            gz = sbg.tile([128, 4, RW], F8, tag="g", name=f"gz{i}")
            nc.vector.memset(gz[:], 0.0)

        def emit_pool(lp, h):
            """Mean-pool layer lp's output h (feature-major) into OUT."""
            poolp = ps.tile([8, 512], F32, tag="stat", bufs=1,
                            name=f"poolp{lp}")
            for m in range(NT):
                hnm = sbm.tile([128, 512], BF, tag="hnm", bufs=2,
                               name=f"hnm{lp}_{m}")
                for kc in range(4):
                    tp2 = ps.tile([128, 128], BF, tag="tp2", bufs=1,
                                  name=f"tp2{lp}_{m}_{kc}")
                    nc.tensor.transpose(
                        tp2[:], h[:, kc, m * 128:(m + 1) * 128], identb[:])
                    nc.scalar.activation(hnm[:, kc * 128:(kc + 1) * 128],
                                         tp2[:], AF.Copy)
                nc.tensor.matmul(poolp[:],
                                 lhsT=spool_sb[:, m * GPC:(m + 1) * GPC],
                                 rhs=hnm[:], start=(m == 0),
                                 stop=(m == NT - 1))
            pool_sb = sbs.tile([GPC, 512], F32, tag="poolsb", bufs=2,
                               name=f"pool{lp}")
            nc.scalar.activation(pool_sb[:], poolp[:], AF.Identity,
                                 scale=cntr_sb[:, 0:1])
            nc.sync.dma_start(out=OUT[:, lp * 512:(lp + 1) * 512],
                              in_=pool_sb[:])

        for l in range(L):
            KIN = 1 if l == 0 else 4

            w_sb = sbw.tile([128, 4 * KIN, 512], BF, tag="w", name=f"w{l}")
            if l == 0:
                nc.sync.dma_start(
                    out=w_sb[:], in_=W0[:, :].rearrange("p (c n) -> p c n", c=4))
            else:
                nc.sync.dma_start(
                    out=w_sb[:],
                    in_=WR[(l - 1) * 2048: l * 2048, :].rearrange(
                        "(c p) n -> p c n", p=128))

            bias_sb = sbs.tile([1, 2048], BF, tag="bias", bufs=1,
                               name=f"bias{l}")
            nc.sync.dma_start(out=bias_sb[:],
                              in_=BIAS[0:1, l * 2048:(l + 1) * 2048])
            gam_sb = sbs.tile([1, 512], F32, tag="gam", bufs=1, name=f"gam{l}")
            nc.sync.dma_start(out=gam_sb[:], in_=GAM[0:1, l * 512:(l + 1) * 512])
            bet_sb = sbs.tile([1, 512], F32, tag="bet", bufs=1, name=f"bet{l}")
            nc.sync.dma_start(out=bet_sb[:], in_=BET[0:1, l * 512:(l + 1) * 512])

            kv_loc = dram.tile([NLOC, RW], F8, tag="kvloc", name=f"kvloc{l}")
            kv_full = dram.tile([KVROWS, RW], F8, tag="kvfull",
                                addr_space="Shared", name=f"kvfull{l}")

            # -- phase A1: K,V projections -> kv_loc; AllGather in two halves
            # so the first collective overlaps the second half's projections
            for m in range(NT):
                kv_sb = sbm.tile([128, RW], F8, tag="kvp", bufs=2,
                                 name=f"kv{l}_{m}")
                for pr in (1, 2):  # 1=k 2=v
                    pp = ps.tile([128, 2, 512], F32, tag="qd", bufs=2,
                                 name=f"pp{l}_{m}_{pr}")
                    for kc in range(KIN):
                        nc.tensor.matmul(
                            pp[:, 0, :], lhsT=h_cur[:, kc, m * 128:(m + 1) * 128],
                            rhs=w_sb[:, pr * KIN + kc, :],
                            start=(kc == 0), stop=False)
                    nc.tensor.matmul(
                        pp[:, 0, :], lhsT=ones1[:],
                        rhs=bias_sb[0:1, pr * 512:(pr + 1) * 512],
                        start=False, stop=True)
                    if pr == 1:   # K half, bf16
                        nc.scalar.activation(
                            kv_sb[:, 0:1024].bitcast(BF), pp[:, 0, :], AF.Copy)
                    else:         # V half, fp8e4
                        nc.scalar.activation(
                            kv_sb[:, 1024:RW], pp[:, 0, :], AF.Copy)
                nc.sync.dma_start(out=kv_loc[m * 128:(m + 1) * 128, :],
                                  in_=kv_sb[:])

            nc.gpsimd.collective_compute(
                "AllGather", mybir.AluOpType.bypass, replica_groups=rg,
                ins=[kv_loc[:].opt()], outs=[kv_full[:].opt()])

            # previous layer's pooling, deferred into the AllGather window
            if l > 0:
                emit_pool(l - 1, h_cur)

            # -- phase A2 (overlaps AllGather): Q + root projections
            Q_sb = sbm.tile([128, NT, 512], BF, tag="q", bufs=1, name=f"q{l}")
            pre_sb = sbm.tile([128, NT, 512], F32, tag="pre", bufs=1,
                              name=f"pre{l}")
            for m in range(NT):
                for pr in (0, 3):  # 0=q 3=root
                    pp = ps.tile([128, 2, 512], F32, tag="qd", bufs=2,
                                 name=f"qr{l}_{m}_{pr}")
                    for kc in range(KIN):
                        nc.tensor.matmul(
                            pp[:, 0, :], lhsT=h_cur[:, kc, m * 128:(m + 1) * 128],
                            rhs=w_sb[:, pr * KIN + kc, :],
                            start=(kc == 0), stop=False)
                    nc.tensor.matmul(
                        pp[:, 0, :], lhsT=ones1[:],
                        rhs=bias_sb[0:1, pr * 512:(pr + 1) * 512],
                        start=False, stop=True)
                    if pr == 0:
                        nc.scalar.activation(Q_sb[:, m, :], pp[:, 0, :], AF.Copy)
                    else:
                        nc.scalar.activation(pre_sb[:, m, :], pp[:, 0, :],
                                             AF.Copy)

            # -- phase B: edge stage
            # rows 0 / 32: sum / sum-of-squares (matmul out base partition
            # must be 0, 32, or 64)
            stat_ps = ps.tile([33, 512], F32, tag="stat", bufs=1,
                              name=f"stat{l}")
            ch0 = 0
            ga0 = 0
            for m in range(NT):
                # stream both indicator orientations for this block (tiny,
                # on the otherwise-idle regular DMA queues)
                stb = sbs.tile([128, CH[m] * 128], BF, tag="stb", bufs=2,
                               name=f"stb{l}_{m}")
                nc.sync.dma_start(
                    out=stb[:], in_=STC[:, ch0 * 128:(ch0 + CH[m]) * 128])
                sttb = sbs.tile([128, CH[m] * 128], BF, tag="sttb", bufs=2,
                                name=f"sttb{l}_{m}")
                nc.sync.dma_start(
                    out=sttb[:], in_=STT[:, ch0 * 128:(ch0 + CH[m]) * 128])
                acc = ps.tile([128, 512], F32, tag="acc", bufs=1,
                              name=f"acc{l}_{m}")
                den = ps.tile([128, 8], F32, tag="den", bufs=1,
                              name=f"den{l}_{m}")
                for g in range(GB[m]):
                    nch = min(4, CH[m] - 4 * g)
                    gt = sbg.tile([128, 4, RW], F8, tag="g",
                                  name=f"gt{l}_{m}_{g}")
                    ga = ga0 + g
                    nidx = nch * 128  # partial tail gathers move fewer rows
                    nc.gpsimd.dma_gather(
                        gt[:, 0:nch, :], kv_full[:, :],
                        idx_sb[:, ga * (GI // 16):
                               ga * (GI // 16) + nidx // 16],
                        nidx, nidx, RW)
                    lg = sbs.tile([128, 4, 8], F32, tag="lg", bufs=4,
                                  name=f"lg{l}_{ga}")
                    pbf = sbs.tile([128, 4, 8], BF, tag="p", bufs=4,
                                   name=f"p{l}_{ga}")
                    pv = sbm.tile([128, 4, 512], BF, tag="pv", bufs=2,
                                  name=f"pv{l}_{ga}")
                    for g2 in range((nch + 1) // 2):
                        n2 = min(2, nch - 2 * g2)
                        qd = ps.tile([128, 2, 512], F32, tag="qd", bufs=2,
                                     name=f"qd{l}_{ga}_{g2}")
                        for i in range(n2):
                            ci = 4 * g + 2 * g2 + i
                            nc.tensor.matmul(
                                qd[:, i, :],
                                lhsT=sttb[:, ci * 128:(ci + 1) * 128],
                                rhs=Q_sb[:, m, :], start=True, stop=True)
                        nc.scalar.activation(pv[:, 2 * g2:2 * g2 + n2, :],
                                             qd[:, 0:n2, :], AF.Copy)
                    pvv = pv[:, 0:nch, :]
                    nc.vector.tensor_mul(pvv, pvv,
                                         gt[:, 0:nch, 0:1024].bitcast(BF))
                    nc.vector.tensor_reduce(
                        lg[:, 0:nch, :],
                        pvv.rearrange("p n (h c) -> p n h c", h=8),
                        mybir.AxisListType.X, mybir.AluOpType.add)
                    nc.scalar.activation(pbf[:, 0:nch, :], lg[:, 0:nch, :],
                                         AF.Exp, scale=0.125)
                    nc.vector.tensor_mul(
                        pv[:, 0:nch, :].rearrange("p n (h c) -> p n h c", h=8),
                        gt[:, 0:nch, 1024:RW].rearrange(
                            "p n (h c) -> p n h c", h=8),
                        pbf[:, 0:nch, :, None].broadcast_to([128, nch, 8, 64]))
                    for cc in range(nch):
                        ci = 4 * g + cc
                        first = (ci == 0)
                        last = (ci == CH[m] - 1)
                        nc.tensor.matmul(acc[:],
                                         lhsT=stb[:, ci * 128:(ci + 1) * 128],
                                         rhs=pv[:, cc, :],
                                         start=first, stop=last)
                        nc.tensor.matmul(den[:],
                                         lhsT=stb[:, ci * 128:(ci + 1) * 128],
                                         rhs=pbf[:, cc, :],
                                         start=first, stop=last)

                # block finalize: normalize, add root (staged in pre_sb), stats
                dsb = sbs.tile([128, 8], F32, tag="dsb", name=f"dsb{l}_{m}")
                nc.scalar.activation(dsb[:], den[:], AF.Copy, bias=1e-16)
                rec = sbs.tile([128, 8], F32, tag="rec", name=f"rec{l}_{m}")
                nc.vector.reciprocal(rec[:], dsb[:])
                msgt = sbm.tile([128, 512], F32, tag="msg", bufs=2,
                                name=f"msg{l}_{m}")
                nc.vector.tensor_mul(
                    msgt[:].rearrange("p (h c) -> p h c", h=8),
                    acc[:].rearrange("p (h c) -> p h c", h=8),
                    rec[:, :, None].broadcast_to([128, 8, 64]))
                nc.vector.tensor_add(pre_sb[:, m, :], msgt[:], pre_sb[:, m, :])
                sq = sbm.tile([128, 512], F32, tag="sq", bufs=2,
                              name=f"sq{l}_{m}")
                nc.scalar.activation(sq[:], pre_sb[:, m, :], AF.Square)
                nc.tensor.matmul(stat_ps[0:1, :], lhsT=mask_sb[:, m:m + 1],
                                 rhs=pre_sb[:, m, :], start=(m == 0),
                                 stop=(m == NT - 1), skip_group_check=True)
                nc.tensor.matmul(stat_ps[32:33, :], lhsT=mask_sb[:, m:m + 1],
                                 rhs=sq[:], start=(m == 0),
                                 stop=(m == NT - 1), skip_group_check=True)
                ch0 += CH[m]
                ga0 += GB[m]

            # -- BN stats AllReduce
            statacc = sbs.tile([1, 1024], F32, tag="statacc", bufs=2,
                               name=f"statacc{l}")
            nc.vector.tensor_copy(out=statacc[0:1, 0:512], in_=stat_ps[0:1, :])
            nc.vector.tensor_copy(out=statacc[0:1, 512:1024],
                                  in_=stat_ps[32:33, :])
            arin = dram.tile([1, 1024], F32, tag="arin", name=f"arin{l}")
            arout_d = dram.tile([1, 1024], F32, tag="arout",
                                addr_space="Shared", name=f"arout{l}")
            nc.sync.dma_start(out=arin[:], in_=statacc[:])
            nc.gpsimd.collective_compute(
                "AllReduce", mybir.AluOpType.add, replica_groups=rg,
                ins=[arin[:].opt()], outs=[arout_d[:].opt()])
            aro = sbs.tile([1, 1024], F32, tag="aro", bufs=1, name=f"aro{l}")
            nc.sync.dma_start(out=aro[:], in_=arout_d[:])

            # A = gamma * rstd ; Bb = beta - mu * A   (rows: [A; Bb])
            mu = sbs.tile([1, 512], F32, tag="mu", bufs=1, name=f"mu{l}")
            nc.scalar.activation(mu[:], aro[0:1, 0:512], AF.Copy, scale=1.0 / N)
            ex2 = sbs.tile([1, 512], F32, tag="ex2", bufs=1, name=f"ex2{l}")
            nc.scalar.activation(ex2[:], aro[0:1, 512:1024], AF.Copy,
                                 scale=1.0 / N)
            var = sbs.tile([1, 512], F32, tag="var", bufs=1, name=f"var{l}")
            nc.vector.tensor_mul(var[:], mu[:], mu[:])
            nc.vector.tensor_sub(var[:], ex2[:], var[:])
            stdt = sbs.tile([1, 512], F32, tag="stdt", bufs=1, name=f"stdt{l}")
            nc.scalar.activation(stdt[:], var[:], AF.Sqrt, bias=BN_EPS)
            rstd = sbs.tile([1, 512], F32, tag="rstd", bufs=1, name=f"rstd{l}")
            nc.vector.reciprocal(rstd[:], stdt[:])
            ab = sbs.tile([2, 512], F32, tag="ab", bufs=1, name=f"ab{l}")
            nc.vector.tensor_mul(ab[0:1, :], gam_sb[0:1, :], rstd[:])
            tmB = sbs.tile([1, 512], F32, tag="tmB", bufs=1, name=f"tmB{l}")
            nc.vector.tensor_mul(tmB[:], mu[:], ab[0:1, :])
            bbrow = sbs.tile([1, 512], F32, tag="bbrow", bufs=1,
                             name=f"bbrow{l}")
            nc.vector.tensor_sub(bbrow[:], bet_sb[0:1, :], tmB[:])
            nc.sync.dma_start(out=ab[1:2, :], in_=bbrow[:])

            abT = sbs.tile([128, 4, 2], F32, tag="abT", name=f"abT{l}")
            for kc in range(4):
                tp = ps.tile([128, 2], F32, tag="den", bufs=1,
                             name=f"abtp{l}_{kc}")
                nc.tensor.transpose(tp[:], ab[:, kc * 128:(kc + 1) * 128],
                                    identf[0:2, 0:2])
                nc.vector.tensor_copy(out=abT[:, kc, :], in_=tp[:])

            # -- h_next = relu(A*pre + Bb) in feature-major
            h_nxt = sbh.tile([128, 4, NLOC], BF, tag="h", name=f"h{l + 1}")
            for m in range(NT):
                for kc in range(4):
                    tp1 = ps.tile([128, 128], F32, tag="qd", bufs=2,
                                  name=f"tp1{l}_{m}_{kc}")
                    nc.tensor.transpose(
                        tp1[:], pre_sb[:, m, kc * 128:(kc + 1) * 128],
                        identf[:])
                    nc.scalar.activation(
                        h_nxt[:, kc, m * 128:(m + 1) * 128], tp1[:], AF.Relu,
                        scale=abT[:, kc, 0:1], bias=abT[:, kc, 1:2])

            h_cur = h_nxt

        # pool for the last layer (earlier layers pooled inside the loop,
        # overlapped with the next layer's AllGather)
        emit_pool(L - 1, h_cur)

    return nc


def _host_shard(x, edge_index, batch):
    """Build all per-core host-side index/constant arrays with tight
    per-block chunk packing (counts maxed over cores for SPMD)."""
    batch = np.asarray(batch)
    src = np.asarray(edge_index[0])
    dst = np.asarray(edge_index[1])
    n = x.shape[0]

    node_start = np.searchsorted(batch, np.arange(0, B, GPC))
    node_end = np.searchsorted(batch, np.arange(GPC - 1, B, GPC), side="right")
    nloc = node_end - node_start
    NT = int(-(-nloc.max() // 128))
    NLOC = NT * 128

    core_of_node = batch // GPC
    local_of_node = np.arange(n) - node_start[core_of_node]
    grow_of_node = core_of_node * NLOC + local_of_node

    ec = core_of_node[dst]
    ld = local_of_node[dst]

    # per-(core,block) edge counts -> per-block chunk counts (max over cores)
    counts = np.zeros((NCORE, NT), np.int64)
    for c in range(NCORE):
        m = ec == c
        counts[c] = np.bincount(ld[m] // 128, minlength=NT)
    CH = [max(1, int(v)) for v in (-(-counts.max(axis=0) // 128))]
    GB = [int(-(-chm // 4)) for chm in CH]
    CHT = sum(CH)
    NGA = sum(GB)

    idx16 = np.full((NCORE, 128, NGA * (GI // 16)), -1, np.int16)
    stc = np.zeros((NCORE, 128, CHT * 128), np.float32)
    stt = np.zeros((NCORE, 128, CHT * 128), np.float32)
    mask = np.zeros((NCORE, 128, NT), np.float32)
    spool = np.zeros((NCORE, 128, NT * GPC), np.float32)
    cntr = np.zeros((NCORE, GPC, 1), np.float32)
    xT = np.zeros((NCORE, 128, NLOC), np.float32)

    jj = np.arange(128)
    x = np.asarray(x)
    for c in range(NCORE):
        ns, nl = node_start[c], nloc[c]
        xT[c, :, :nl] = x[ns:ns + nl].T
        m2 = np.zeros(NLOC, np.float32)
        m2[:nl] = 1.0
        mask[c] = m2.reshape(NT, 128).T
        gl = batch[ns:ns + nl] - c * GPC
        sp = np.zeros((NLOC, GPC), np.float32)
        sp[np.arange(nl), gl] = 1.0
        spool[c] = sp.reshape(NT, 128, GPC).transpose(1, 0, 2).reshape(
            128, NT * GPC)
        cnt = sp.sum(axis=0)
        cntr[c, :, 0] = 1.0 / np.maximum(cnt, 1.0)

        eids = np.nonzero(ec == c)[0]
        order = np.argsort(ld[eids], kind="stable")
        eids = eids[order]
        lds = ld[eids]
        srows = grow_of_node[src[eids]]
        blk = lds // 128
        bc = np.bincount(blk, minlength=NT)
        pos = 0
        ch0 = 0
        ga0 = 0
        for m in range(NT):
            n_ = int(bc[m])
            nslot = GB[m] * 512
            # pad slots gather row 0 (negative "skip" indices hang the
            # gather ucode on this runtime); dst -1 keeps the indicator
            # column zero so they contribute nothing
            a_src = np.zeros(nslot, np.int64)
            a_dst = np.full(nslot, -1.0, np.float32)
            a_src[:n_] = srows[pos:pos + n_]
            a_dst[:n_] = (lds[pos:pos + n_] % 128).astype(np.float32)
            pos += n_
            # gather indices: idx i of gather g -> partition i%16, col i//16
            w = a_src.reshape(GB[m], GI // 16, 16)
            wt = w.transpose(0, 2, 1).reshape(GB[m], 16, GI // 16)
            for g in range(GB[m]):
                cols = slice((ga0 + g) * (GI // 16), (ga0 + g + 1) * (GI // 16))
                for r in range(8):
                    idx16[c, r * 16:(r + 1) * 16, cols] = wt[g]
            # per-chunk indicator matrices, both orientations
            for ci in range(CH[m]):
                col = a_dst[ci * 128:(ci + 1) * 128]
                sl = slice((ch0 + ci) * 128, (ch0 + ci + 1) * 128)
                stc[c, :, sl] = (col[:, None] == jj[None, :]).astype(
                    np.float32)
                stt[c, :, sl] = (col[None, :] == jj[:, None]).astype(
                    np.float32)
            ch0 += CH[m]
            ga0 += GB[m]

    return (NT, CH, GB, node_start, idx16, stc, stt, mask, spool, cntr, xT)


def kernel(x, edge_index, batch, W0_q, b0_q, W0_k, b0_k, W0_v, b0_v,
           W0_s, b0_s, Wq, bq, Wk, bk, Wv, bv, Ws, bs, gamma, beta):
    from concourse.bass_utils import run_bass_kernel_spmd

    (NT, CH, GB, node_start, idx16, stc, stt, mask, spool, cntr, xT) = \
        _host_shard(x, edge_index, batch)

    W0a = np.concatenate([np.asarray(W0_q), np.asarray(W0_k),
                          np.asarray(W0_v), np.asarray(W0_s)], axis=1)
    WRa = np.zeros((7 * 2048, 512), np.float32)
    Wstack = [np.asarray(Wq), np.asarray(Wk), np.asarray(Wv), np.asarray(Ws)]
    for li in range(7):
        for pr in range(4):
            for kc in range(4):
                r0 = li * 2048 + pr * 512 + kc * 128
                WRa[r0:r0 + 128] = Wstack[pr][li][kc * 128:(kc + 1) * 128, :]
    BIASa = np.zeros((8, 2048), np.float32)
    BIASa[0] = np.concatenate([np.asarray(b0_q), np.asarray(b0_k),
                               np.asarray(b0_v), np.asarray(b0_s)])
    bstack = [np.asarray(bq), np.asarray(bk), np.asarray(bv), np.asarray(bs)]
    for li in range(7):
        BIASa[li + 1] = np.concatenate([bstack[pr][li] for pr in range(4)])

    iota_f = np.tile(np.arange(128, dtype=np.float32)[None, :], (128, 1))
    ones1 = np.ones((1, 128), np.float32)
    ident = np.eye(128, dtype=np.float32)

    common = {
        "W0": _to_bf(W0a), "WR": _to_bf(WRa),
        "BIAS": _to_bf(BIASa.reshape(1, -1)),
        "GAM": np.asarray(gamma, np.float32).reshape(1, -1),
        "BET": np.asarray(beta, np.float32).reshape(1, -1),
        "ONES1": _to_bf(ones1), "IDENTF": ident, "IDENTB": _to_bf(ident),
    }
    in_maps = []
    for c in range(NCORE):
        in_maps.append(dict(
            common,
            XT=_to_bf(xT[c]), IDX=idx16[c],
            STC=_to_bf(stc[c]), STT=_to_bf(stt[c]),
            MASK=mask[c], SPOOL=_to_bf(spool[c]), CNTR=cntr[c],
        ))

    nc = _build_nc(NT, CH, GB)
    nc.compile()
    res = run_bass_kernel_spmd(nc, in_maps, list(range(NCORE)))
    out = np.zeros((B, L * 512), np.float32)
    for c in range(NCORE):
        out[c * GPC:(c + 1) * GPC] = res.results[c]["POOLED"]
    return out


if __name__ == "__main__":
    pass
